# revision 6
# baseline (speedup 1.0000x reference)
"""Trainium2 Bass kernel for nn_Attention_33646773797316.

Math: the reference's 4-layer MLP has no activations, so everything after the
softmax collapses:
    w[g,m] = (sum_n attn[g,m,n] * u[g,n]) + bmlp,   u = factors @ (Wv @ W1@W2@W3@W4)
    scores = factors @ A @ factors.T,               A = Wq @ Wk.T
    out[n,g] = sum_m raw[n,g,m] * w[g,m] * valid[g,m]
The heavy part is the last contraction over raw (205 MB).

Strategy: data-parallel over N across 8 cores.  Host pre-transposes each raw
shard to [G*M, n] layout so the big contraction runs on the TensorEngine as 8
PSUM-accumulated matmuls per 512-column block, with the (tiny) attention
pipeline computed on-device per core and folded into block-diagonal stationary
weight matrices.  float32r matmuls for the big contraction (full PE speed),
plain fp32 for the attention/scores (softmax is tie-sensitive).
"""

import sys
import types

sys.path.insert(0, "/opt/trn_rl_repo")

import numpy as np

N, G, M, F, D = 50000, 64, 16, 256, 512
NCORES = 8
NSH = N // NCORES  # 6250 rows per core
NB = 512  # n-block width for the main contraction
NEG = -1.0e30

TRACE = False  # set by test.py to collect a profile
LAST_RESULTS = None
LAST_EXEC_NS = None

_prog_cache = {}


def _ensure_axon_hooks():
    """Provide antenv.axon_hooks + the NTFF profile hook (for TRACE mode)."""
    try:
        import antenv
    except ImportError:
        return
    if "antenv.axon_hooks" not in sys.modules:
        m = types.ModuleType("antenv.axon_hooks")
        m._hook = None
        m.set_axon_ntff_profile_hook = lambda h, _m=m: setattr(_m, "_hook", h)
        m.get_axon_ntff_profile_hook = lambda _m=m: _m._hook
        sys.modules["antenv.axon_hooks"] = m
        antenv.axon_hooks = m
    if sys.modules["antenv.axon_hooks"]._hook is None:
        try:
            from trn_agent_boot.trn_boot import _ntff_profile_via_ctypes

            hk = _ntff_profile_via_ctypes("/opt/axon/libaxon_pjrt.so")
            if hk is not None:
                sys.modules["antenv.axon_hooks"].set_axon_ntff_profile_hook(hk)
        except Exception:
            pass


def _build_program():
    if "nc" in _prog_cache:
        return _prog_cache["nc"]

    import concourse.bacc as bacc
    import concourse.mybir as mybir
    import concourse.tile as tile

    f32 = mybir.dt.float32
    bf16 = mybir.dt.bfloat16
    Act = mybir.ActivationFunctionType
    Alu = mybir.AluOpType
    Ax = mybir.AxisListType

    nc = bacc.Bacc("TRN2", target_bir_lowering=False, debug=False, num_devices=NCORES)

    raw_t = nc.declare_dram_parameter("raw_t", [128, 8, NSH], f32, isOutput=False)
    fct = nc.declare_dram_parameter("factors_t", [256, 1024], f32, isOutput=False)
    amat = nc.declare_dram_parameter("amat", [256, 256], f32, isOutput=False)
    wv = nc.declare_dram_parameter("wv", [256, 1], f32, isOutput=False)
    madd = nc.declare_dram_parameter("madd", [128, 8, 128], f32, isOutput=False)
    emask = nc.declare_dram_parameter("emask", [128, 8, 64], f32, isOutput=False)
    bconst = nc.declare_dram_parameter("bconst", [128, 1], f32, isOutput=False)
    ident = nc.declare_dram_parameter("ident", [128, 128], f32, isOutput=False)
    out_t = nc.declare_dram_parameter("out", [64, NSH], f32, isOutput=True)

    nblocks = (NSH + NB - 1) // NB

    with tile.TileContext(nc) as tc:
        with (
            tc.tile_pool(name="const", bufs=1) as cpool,
            tc.tile_pool(name="work", bufs=2) as wpool,
            tc.tile_pool(name="raw", bufs=4) as rpool,
            tc.tile_pool(name="obuf", bufs=3) as opool,
            tc.tile_pool(name="psA", bufs=3, space="PSUM") as psA,
            tc.tile_pool(name="psB", bufs=1, space="PSUM") as psB,
            tc.tile_pool(name="psO", bufs=3, space="PSUM") as psO,
        ):
            # ---------------- constants into SBUF ----------------
            ft = cpool.tile([128, 2, 1024], f32)  # factors.T, f-chunk major
            nc.sync.dma_start(ft[:, 0, :], fct[0:128, :])
            nc.sync.dma_start(ft[:, 1, :], fct[128:256, :])
            a_sb = cpool.tile([128, 4, 128], f32)  # A blocks (fi, fo)
            for fi in range(2):
                for fo in range(2):
                    nc.sync.dma_start(
                        a_sb[:, fi * 2 + fo, :],
                        amat[fi * 128 : (fi + 1) * 128, fo * 128 : (fo + 1) * 128],
                    )
            wv_sb = cpool.tile([128, 2], f32)
            nc.sync.dma_start(wv_sb[:, 0:1], wv[0:128, :])
            nc.sync.dma_start(wv_sb[:, 1:2], wv[128:256, :])
            md_sb = cpool.tile([128, 8, 128], f32)
            nc.sync.dma_start(md_sb[:, :, :], madd[:, :, :])
            em_sb = cpool.tile([128, 8, 64], f32)
            nc.sync.dma_start(em_sb[:, :, :], emask[:, :, :])
            bc_sb = cpool.tile([128, 1], f32)
            nc.sync.dma_start(bc_sb[:, :], bconst[:, :])
            id_sb = cpool.tile([128, 128], f32)
            nc.sync.dma_start(id_sb[:, :], ident[:, :])

            # ---------------- fAT = (factors @ A).T ----------------
            # layout [f2-chunk (2) x 128 partitions, 1024 tokens]
            fa_sb = cpool.tile([128, 2, 1024], f32)
            for fo in range(2):
                for th in range(2):
                    pfa = psA.tile([128, 512], f32, tag="psA")
                    for fi in range(2):
                        nc.tensor.matmul(
                            pfa[:, :],
                            a_sb[:, fi * 2 + fo, :],
                            ft[:, fi, th * 512 : (th + 1) * 512],
                            start=(fi == 0),
                            stop=(fi == 1),
                        )
                    nc.scalar.copy(fa_sb[:, fo, th * 512 : (th + 1) * 512], pfa[:, :])

            # ---------------- u = factors @ wv, per token chunk ----------------
            pu = psB.tile([128, 8], f32, tag="pu")
            for c in range(8):
                for fi in range(2):
                    nc.tensor.matmul(
                        pu[:, c : c + 1],
                        ft[:, fi, c * 128 : (c + 1) * 128],
                        wv_sb[:, fi : fi + 1],
                        start=(fi == 0),
                        stop=(fi == 1),
                    )
            u_sb = cpool.tile([128, 8], f32)
            nc.vector.tensor_copy(u_sb[:, :], pu[:, :])

            # ---------------- masked softmax numerator/denominator ----------------
            s0 = cpool.tile([128, 8], f32)  # sum of exp, per chunk column
            s1 = psB.tile([128, 8], f32, tag="s1")  # sum of exp * u
            for c in range(8):
                ps_s = psA.tile([128, 128], f32, tag="psA")
                for fo in range(2):
                    nc.tensor.matmul(
                        ps_s[:, :],
                        fa_sb[:, fo, c * 128 : (c + 1) * 128],
                        ft[:, fo, c * 128 : (c + 1) * 128],
                        start=(fo == 0),
                        stop=(fo == 1),
                    )
                sc = wpool.tile([128, 128], f32, tag="sc")
                nc.vector.tensor_tensor(sc[:, :], ps_s[:, :], md_sb[:, c, :], op=Alu.add)
                mx = wpool.tile([128, 1], f32, tag="mx")
                nc.vector.tensor_reduce(
                    mx[:, :], sc[:, :], axis=Ax.X, op=Alu.max, negate=True
                )
                e = wpool.tile([128, 128], f32, tag="e")
                nc.scalar.activation(
                    e[:, :],
                    sc[:, :],
                    Act.Exp,
                    bias=mx[:, 0:1],
                    scale=1.0,
                    accum_out=s0[:, c : c + 1],
                )
                peT = psA.tile([128, 128], f32, tag="psA")
                nc.tensor.transpose(peT[:, :], e[:, :], id_sb[:, :])
                eT = wpool.tile([128, 128], f32, tag="eT")
                nc.scalar.copy(eT[:, :], peT[:, :])
                nc.tensor.matmul(
                    s1[:, c : c + 1], eT[:, :], u_sb[:, c : c + 1], start=True, stop=True
                )

            # w = s1/s0 + bmlp ; stationaries W64_c = emask_c * w_col_c
            r0 = cpool.tile([128, 8], f32)
            nc.vector.reciprocal(r0[:, :], s0[:, :])
            wq = cpool.tile([128, 8], f32)
            nc.vector.tensor_tensor(wq[:, :], s1[:, :], r0[:, :], op=Alu.mult)
            wcol = cpool.tile([128, 8], f32)
            nc.vector.tensor_scalar_add(wcol[:, :], wq[:, :], bc_sb[:, 0:1])
            wstat = cpool.tile([128, 8, 64], bf16)
            for c in range(8):
                nc.vector.tensor_scalar_mul(
                    wstat[:, c, :], em_sb[:, c, :], wcol[:, c : c + 1]
                )

            # ---------------- main contraction over raw ----------------
            # f32 HWDGE DMA in, on-chip cast to bf16 (alternating DVE/ACT),
            # bf16 matmuls at full PE rate.
            for b in range(nblocks):
                b0 = b * NB
                nb = min(NB, NSH - b0)
                rt = rpool.tile([128, 8, nb], f32, tag="rt")
                nc.sync.dma_start(rt[:, :, :], raw_t[:, :, b0 : b0 + nb])
                rtb = rpool.tile([128, 8, nb], bf16, tag="rtb")
                if b % 2 == 0:
                    nc.vector.tensor_copy(rtb[:, :, :], rt[:, :, :])
                else:
                    nc.scalar.copy(rtb[:, :, :], rt[:, :, :])
                po = psO.tile([64, nb], f32, tag="po")
                for c in range(8):
                    nc.tensor.matmul(
                        po[:, :],
                        wstat[:, c, :],
                        rtb[:, c, :],
                        start=(c == 0),
                        stop=(c == 7),
                    )
                ob = opool.tile([64, nb], f32, tag="ob")
                nc.vector.tensor_copy(ob[:, :], po[:, :])
                nc.sync.dma_start(out_t[:, b0 : b0 + nb], ob[:, :])

    nc.compile()
    _prog_cache["nc"] = nc
    return nc


def kernel(**inputs):
    global LAST_RESULTS, LAST_EXEC_NS
    _ensure_axon_hooks()
    from concourse.bass_utils import run_bass_kernel_spmd

    raw = np.ascontiguousarray(np.asarray(inputs["raw"], dtype=np.float32))
    factors = np.asarray(inputs["factors"], dtype=np.float32)
    lengths = np.asarray(inputs["lengths"], dtype=np.int32)
    Wq = np.asarray(inputs["Wq"], dtype=np.float32)
    Wk = np.asarray(inputs["Wk"], dtype=np.float32)
    Wv = np.asarray(inputs["Wv"], dtype=np.float32)
    W1 = np.asarray(inputs["W1"], dtype=np.float32)
    b1 = np.asarray(inputs["b1"], dtype=np.float32)
    W2 = np.asarray(inputs["W2"], dtype=np.float32)
    b2 = np.asarray(inputs["b2"], dtype=np.float32)
    W3 = np.asarray(inputs["W3"], dtype=np.float32)
    b3 = np.asarray(inputs["b3"], dtype=np.float32)
    W4 = np.asarray(inputs["W4"], dtype=np.float32)
    b4 = np.asarray(inputs["b4"], dtype=np.float32)

    # ----- fold the linear tail on the host (weight-only refactoring) -----
    A = (Wq.astype(np.float64) @ Wk.astype(np.float64).T).astype(np.float32)
    chain = (
        W1.astype(np.float64)
        @ W2.astype(np.float64)
        @ W3.astype(np.float64)
        @ W4.astype(np.float64)
    )  # [D, 1]
    wvv = (Wv.astype(np.float64) @ chain).astype(np.float32)  # [F, 1]
    bmlp = float(
        (
            ((b1.astype(np.float64) @ W2.astype(np.float64) + b2) @ W3.astype(np.float64) + b3)
            @ W4.astype(np.float64)
            + b4
        )[0]
    )

    # ----- masks from lengths -----
    gs = np.arange(128) // 16  # local group of partition p
    mm = np.arange(128) % 16  # local m of partition p

    madd = np.empty((128, 8, 128), dtype=np.float32)
    emask = np.zeros((128, 8, 64), dtype=np.float32)
    for c in range(8):
        g_of_q = 8 * c + gs  # [128] global group of key token q
        valid_q = mm < lengths[g_of_q]  # [128] key validity
        same_g = gs[:, None] == gs[None, :]  # [128, 128]
        madd[:, c, :] = np.where(same_g & valid_q[None, :], 0.0, NEG)
        g_of_p = 8 * c + gs
        row_valid = mm < lengths[g_of_p]
        emask[np.arange(128), c, g_of_p] = row_valid.astype(np.float32)

    bconst = np.full((128, 1), bmlp, dtype=np.float32)
    identity = np.eye(128, dtype=np.float32)
    factors_t = np.ascontiguousarray(factors.reshape(G * M, F).T)  # [256, 1024]

    nc = _build_program()

    in_maps = []
    for i in range(NCORES):
        shard = raw.reshape(N, G * M)[i * NSH : (i + 1) * NSH]
        rt = np.ascontiguousarray(
            shard.reshape(NSH, 8, 128).transpose(2, 1, 0)
        )  # [128, 8, NSH]
        in_maps.append(
            dict(
                raw_t=rt,
                factors_t=factors_t,
                amat=A,
                wv=wvv,
                madd=madd,
                emask=emask,
                bconst=bconst,
                ident=identity,
            )
        )

    res = run_bass_kernel_spmd(nc, in_maps, core_ids=list(range(NCORES)), trace=TRACE)
    LAST_RESULTS = res
    LAST_EXEC_NS = res.exec_time_ns

    out = np.empty((N, G), dtype=np.float32)
    for i in range(NCORES):
        out[i * NSH : (i + 1) * NSH, :] = res.results[i]["out"].T
    return out
